# revision 50
# baseline (speedup 1.0000x reference)
"""TRN2 Bass kernel for nn_KNN_model (conv stack + pairwise patch distances).

Strategy (8 NeuronCores, SPMD):
  - Convs sharded over H: each core computes a 40-row slab (32 owned + 4 halo
    each side) through conv+BN+ReLU layers in float32r (TF32-like) on PE.
    3x3 conv = 6 matmul streams per tile: 3 K=128 pairs (top+mid tap rows via a
    partition-shifted slab copy) + 3 K=64 singles (bottom tap row).
  - BN stats (layers 0-2): per-core partial (mean, var) via bn_stats/bn_aggr,
    tiny AllGather + PE ones-matmul reduce, applied fused in one ACT pass.
  - conv3 emitted directly in patch layout: one K=2304 matmul chain (18 paired
    K=128 streams over the 6x6 stride-4 receptive field) -> [32=(c,py,px), 512]
    raw patch components per core.
  - Raw conv3 output AllGathered once ([8,32,512]); BN3 global stats computed
    locally post-gather (no separate stats collective).
  - Distance phase in fp16: p~=fp16(feat), sq=|p~|^2 split hi+lo fp16 so
    D2 = sq_i + sq_j - 2 p~.p~ stays PSD to ~1e-5.  lhsT rows [p~,1,1],
    rhs rows [-2p~, sq_hi, sq_lo]; per-partition ACT bias adds sq_i + eps and
    fuses sqrt + PSUM->SBUF + fp16 cast in ONE pass.  K=18 matmuls packed 4-way
    into PE row groups.
  - Symmetry: each row block computes only a 4608-wide wrapped column band
    (everything else is the transpose of some band entry). Host mirrors.
"""
import numpy as np
import ml_dtypes
import concourse.bacc as bacc
import concourse.bass as bass
import concourse.tile as tile
from concourse import mybir
from concourse.bass import ds
from concourse.bass_utils import run_bass_kernel_spmd

F32 = mybir.dt.float32
F32R = mybir.dt.float32r
F16 = mybir.dt.float16
AF = mybir.ActivationFunctionType
ALU = mybir.AluOpType

NCORES = 8
WP = 258            # padded row width (256 + 2 pad cols)
ROWS = 40           # ext slab rows per core (32 owned + 4 halo each side)
LEAD = 4            # lead margin so tap offsets never go negative
HROWS = 42          # slab rows + 1 pad row top/bottom
HFREE = LEAD + HROWS * WP + 4   # 10844
YFREE = ROWS * WP   # 10320
EPS = 1e-5
EPS2 = 2e-4         # d^2 floor bias before sqrt (abs err ~0.014 at d=0)
GOFF = [0, 64, 128, 192]        # g/be packing offsets per layer
COUT = [64, 64, 64, 2]
BAND = 4608         # symmetric band width (9 x 512)
REXT = 8192 + BAND  # extended rhs width for wraparound

_CACHE = {}


def _conv_tiles(s0=0, s1=YFREE):
    out, s = [], s0
    while s < s1:
        L = min(512, s1 - s)
        out.append((s, L))
        s += L
    return out


def build():
    nc = bacc.Bacc(trn_type="TRN2", num_devices=NCORES)
    x0 = nc.dram_tensor("x0", [27, YFREE], F32, kind="ExternalInput").ap()
    w0T = nc.dram_tensor("w0T", [27, 64], F32, kind="ExternalInput").ap()
    wp_in, ws_in = {}, {}
    for l in (1, 2):
        co = COUT[l]
        wp_in[l] = nc.dram_tensor(f"wp{l}", [3, 128, co], F32, kind="ExternalInput").ap()
        ws_in[l] = nc.dram_tensor(f"ws{l}", [3, 64, co], F32, kind="ExternalInput").ap()
    w3c_in = nc.dram_tensor("w3c", [18, 128, 32], F32, kind="ExternalInput").ap()
    b32f_in = nc.dram_tensor("b32f", [2, 32], F32, kind="ExternalInput").ap()
    b32h_in = nc.dram_tensor("b32h", [32, 2], F16, kind="ExternalInput").ap()
    g_all = nc.dram_tensor("g_all", [1, 194], F32, kind="ExternalInput").ap()
    be_all = nc.dram_tensor("be_all", [1, 194], F32, kind="ExternalInput").ap()
    mask8 = nc.dram_tensor("mask8", [1, 8 * WP], F32, kind="ExternalInput").ap()
    out = nc.dram_tensor("out", [1024, BAND], F16, kind="ExternalOutput").ap()

    TILES = {0: _conv_tiles(WP, 39 * WP),
             1: _conv_tiles(2 * WP, 38 * WP),
             2: _conv_tiles(3 * WP, 37 * WP)}

    with tile.TileContext(nc) as tc:
      with tc.tile_pool(name="pers", bufs=1) as pers, \
           tc.tile_pool(name="dr", bufs=1, space="DRAM") as dr:
        gsb = pers.tile([1, 194], F32)
        nc.sync.dma_start(out=gsb, in_=g_all)
        besb = pers.tile([1, 194], F32)
        nc.sync.dma_start(out=besb, in_=be_all)
        ones1 = pers.tile([1, 1], F32)
        nc.vector.memset(ones1, 1.0)
        ones8 = pers.tile([8, 1], F32)
        nc.vector.memset(ones8, 0.125)   # 1/8 for mean-of-cores matmul
        epst = pers.tile([1, 1], F32)
        nc.vector.memset(epst, EPS)

        def bn_finish(l, C, regions, bnps, sbp):
            """Cross-core BN: partial stats -> AllGather -> scale/shift [C,1]."""
            n = sum(r.shape[1] if r.ndim == 3 else 1 for r in regions)
            st = sbp.tile([C, n, 6], F32, tag=f"st{l}")
            i = 0
            for ap in regions:
                k = ap.shape[1] if ap.ndim == 3 else 1
                o = st[:, i:i + k, :] if ap.ndim == 3 else st[:, i, :]
                nc.vector.bn_stats(out=o, in_=ap)
                i += k
            mvt = sbp.tile([C, 2], F32, tag=f"mv{l}")
            nc.vector.bn_aggr(out=mvt, in_=st)
            sti = dr.tile([C, 2], F32, tag=f"sti{l}")
            sto = dr.tile([NCORES, C, 2], F32, tag=f"sto{l}")
            nc.gpsimd.dma_start(out=sti, in_=mvt)
            nc.gpsimd.collective_compute(
                "AllGather", ALU.bypass,
                replica_groups=[list(range(NCORES))],
                ins=[sti.opt()], outs=[sto.opt()])
            G = sbp.tile([8, 2 * C], F32, tag=f"G{l}")
            nc.sync.dma_start(out=G, in_=sto.rearrange("k c two -> k (c two)"))
            Gv = G.rearrange("p (c two) -> p c two", two=2)
            m2 = sbp.tile([8, C], F32, tag=f"m2{l}")
            nc.vector.tensor_mul(m2, Gv[:, :, 0], Gv[:, :, 0])
            pavg = bnps.tile([1, 2 * C], F32, tag="bn")
            nc.tensor.matmul(pavg, ones8, G, start=True, stop=True)
            pavg2 = bnps.tile([1, C], F32, tag="bn")
            nc.tensor.matmul(pavg2, ones8, m2, start=True, stop=True)
            A1 = sbp.tile([1, 2 * C], F32, tag=f"A1{l}")
            nc.scalar.copy(A1, pavg)
            A2 = sbp.tile([1, C], F32, tag=f"A2{l}")
            nc.scalar.copy(A2, pavg2)
            A1v = A1.rearrange("p (c two) -> p c two", two=2)
            am, av = A1v[:, :, 0], A1v[:, :, 1]
            t1 = sbp.tile([1, C], F32, tag=f"t1{l}")
            nc.vector.tensor_mul(t1, am, am)       # E[m]^2
            t2 = sbp.tile([1, C], F32, tag=f"t2{l}")
            nc.vector.tensor_sub(t2, A2, t1)       # Var(means)
            t3 = sbp.tile([1, C], F32, tag=f"t3{l}")
            nc.vector.tensor_add(t3, t2, av)       # + E[var] = total var
            sd = sbp.tile([1, C], F32, tag=f"sd{l}")
            nc.scalar.activation(sd, t3, AF.Sqrt, bias=epst)
            rs = sbp.tile([1, C], F32, tag=f"rs{l}")
            nc.vector.reciprocal(rs, sd)
            off = GOFF[l]
            scl = sbp.tile([1, C], F32, tag=f"scl{l}")
            nc.vector.tensor_mul(scl, gsb[:, off:off + C], rs)
            sh0 = sbp.tile([1, C], F32, tag=f"sh0{l}")
            nc.vector.tensor_mul(sh0, am, scl)
            sh = sbp.tile([1, C], F32, tag=f"sh{l}")
            nc.vector.tensor_sub(sh, besb[:, off:off + C], sh0)
            psc = bnps.tile([C, 1], F32, tag="bn")
            nc.tensor.matmul(psc, scl, ones1, start=True, stop=True)
            psh = bnps.tile([C, 1], F32, tag="bn")
            nc.tensor.matmul(psh, sh, ones1, start=True, stop=True)
            sbs = sbp.tile([C, 1], F32, tag=f"sbs{l}")
            nc.scalar.copy(sbs, psc)
            sbh = sbp.tile([C, 1], F32, tag=f"sbh{l}")
            nc.scalar.copy(sbh, psh)
            return sbs, sbh

        # ---------------- conv phase ----------------
        y3d = dr.tile([32, 512], F32, tag="y3d")
        with tc.tile_pool(name="cb", bufs=1) as cb, \
             tc.tile_pool(name="hp", bufs=2) as hp, \
             tc.tile_pool(name="cps", bufs=6, space="PSUM") as cps, \
             tc.tile_pool(name="bnps", bufs=2, space="PSUM") as bnps:
            x0t = cb.tile([27, YFREE], F32R)
            nc.gpsimd.dma_start(out=x0t, in_=x0)
            mskf = cb.tile([64, 8 * WP], F32)
            nc.gpsimd.dma_start(out=mskf, in_=mask8.partition_broadcast(64))
            mv_ = mskf.rearrange("p (r c) -> p r c", c=WP)
            w0 = cb.tile([27, 64], F32R)
            nc.gpsimd.dma_start(out=w0, in_=w0T)
            wpair, wsing = {}, {}
            for l in (1, 2):
                co = COUT[l]
                for p in range(3):
                    t = cb.tile([128, co], F32R, tag=f"twp{l}{p}")
                    nc.gpsimd.dma_start(out=t, in_=wp_in[l][p])
                    wpair[(l, p)] = t
                    t2 = cb.tile([64, co], F32R, tag=f"tws{l}{p}")
                    nc.gpsimd.dma_start(out=t2, in_=ws_in[l][p])
                    wsing[(l, p)] = t2
            w3sb = []
            for s in range(18):
                t = cb.tile([128, 32], F32R, tag=f"w3c{s}")
                nc.gpsimd.dma_start(out=t, in_=w3c_in[s])
                w3sb.append(t)

            def finish_layer(l, y):
                """BN + ReLU + mask + padded f32r slab, chunked so the next
                conv's matmuls can start before the whole pass finishes."""
                yv = y.rearrange("p (r c) -> p r c", c=WP)
                regs = [yv[:, r, 1:257] for r in range(4, 36)]
                sbs, sbh = bn_finish(l, 64, regs, bnps, cb)
                h = hp.tile([128, HFREE], F32R, tag="h")
                T0 = LEAD + WP
                nc.vector.memset(h[0:64, 0:T0].bitcast(F32), 0.0)
                nc.vector.memset(h[0:64, T0 + YFREE:HFREE].bitcast(F32), 0.0)
                CH = 10   # slab rows per chunk
                for c0 in range(0, ROWS, CH):
                    a = T0 + c0 * WP
                    nc.scalar.activation(h[0:64, a:a + CH * WP],
                                         y[:, c0 * WP:(c0 + CH) * WP],
                                         AF.Relu, bias=sbh, scale=sbs)
                    hvv = h[0:64, a:a + CH * WP].rearrange(
                        "p (r c) -> p r c", c=WP)
                    if c0 == 0:
                        nc.vector.tensor_mul(hvv[:, 0:4, :], hvv[:, 0:4, :],
                                             mv_[:, 0:4, :])
                    if c0 == 30:
                        nc.vector.tensor_mul(hvv[:, 6:10, :], hvv[:, 6:10, :],
                                             mv_[:, 4:8, :])
                    hcv = h[0:64, a:a + CH * WP].rearrange(
                        "p (r c) -> p c r", c=WP)
                    nc.vector.memset(hcv[:, 0, :].bitcast(F32), 0.0)
                    nc.vector.memset(hcv[:, 257, :].bitcast(F32), 0.0)
                # bottom half = top shifted one row, chunked to follow ACT
                nc.vector.memset(h[64:128, 0:LEAD].bitcast(F32), 0.0)
                nc.vector.memset(h[64:128, LEAD + YFREE:HFREE].bitcast(F32), 0.0)
                for c0 in range(0, ROWS, CH):
                    d0 = LEAD + c0 * WP
                    nc.vector.tensor_copy(h[64:128, d0:d0 + CH * WP],
                                          h[0:64, d0 + WP:d0 + WP + CH * WP])
                return h

            # conv0 (im2col input, K=27, one stream)
            if True:
                y = cb.tile([64, YFREE], F32, tag="y")
                for (s, L) in TILES[0]:
                    ps = cps.tile([64, 512], F32, tag="cps")
                    nc.tensor.matmul(ps[:, 0:L], w0, x0t[:, s:s + L],
                                     start=True, stop=True)
                    nc.scalar.copy(y[:, s:s + L], ps[:, 0:L])
                h = finish_layer(0, y)

            # conv1, conv2 (6 streams: 3 pairs K=128 + 3 singles K=64)
            GROUP = 6
            for l in (1, 2):
                y = cb.tile([64, YFREE], F32, tag="y")
                for g0 in range(0, len(TILES[l]), GROUP):
                    grp = TILES[l][g0:g0 + GROUP]
                    pss = [cps.tile([64, 512], F32, tag="cps", name=f"cps{g0}_{i}")
                           for i in range(len(grp))]
                    for p in range(3):
                        for ps, (s, L) in zip(pss, grp):
                            o = LEAD + 516 + s + p - 1
                            nc.tensor.matmul(ps[:, 0:L], wsing[(l, p)],
                                             h[0:64, o:o + L],
                                             start=(p == 0), stop=False)
                    for p in range(3):
                        for ps, (s, L) in zip(pss, grp):
                            o = LEAD + s + p - 1
                            nc.tensor.matmul(ps[:, 0:L], wpair[(l, p)],
                                             h[0:128, o:o + L],
                                             start=False, stop=(p == 2))
                    for ps, (s, L) in zip(pss, grp):
                        nc.scalar.copy(y[:, s:s + L], ps[:, 0:L])
                h = finish_layer(l, y)

            # conv3 direct to patch layout: out[(c,py,px),(gy,gx)] via K=2304
            # (18 paired K=128 streams over the 6x6 stride-4 window)
            c3ps = cps.tile([64, 512], F32, tag="cps", name="c3ps")[0:32, :]
            si = 0
            for iwyp in range(3):
                wyp = 2 * iwyp
                for wx in range(6):
                    base = LEAD + (wyp + 4) * WP + wx
                    win = h[0:128, base:base + 8 * 4 * WP]
                    w1 = win.rearrange("p (gy r) -> p gy r", gy=8)
                    w2 = w1[:, :, 0:256]
                    rhs = w2.rearrange("p gy (gx s) -> p gy gx s", s=4)[:, :, :, 0:1]
                    nc.tensor.matmul(c3ps, w3sb[si], rhs,
                                     start=(si == 0), stop=(si == 17))
                    si += 1
            y3l = cb.tile([32, 512], F32, tag="y3l")
            nc.scalar.copy(y3l, c3ps)
            nc.gpsimd.dma_start(out=y3d, in_=y3l)

        # ---------------- gather raw conv3 output ----------------
        gath3 = dr.tile([NCORES, 32, 512], F32, tag="gath3")
        nc.gpsimd.collective_compute(
            "AllGather", ALU.bypass,
            replica_groups=[list(range(NCORES))],
            ins=[y3d.opt()], outs=[gath3.opt()])

        # ---------------- BN3 + feature build + distance ----------------
        Rd = dr.tile([18, REXT], F16, tag="Rd")     # extended rhs rows in DRAM
        sqFd = dr.tile([1, 8192], F32, tag="sqFd")  # per-patch |p|^2 (fp32)
        with tc.tile_pool(name="db", bufs=1) as db, \
             tc.tile_pool(name="stg", bufs=2) as stg:
          with tc.tile_pool(name="sps", bufs=2, space="PSUM") as sps:
            G = db.tile([32, 4096], F32)
            nc.sync.dma_start(out=G.rearrange("p (k n) -> p k n", k=8),
                              in_=gath3.rearrange("k p n -> p k n"))

            # global BN3 stats from gathered raw data
            st3 = db.tile([32, 8, 6], F32)
            for j in range(8):
                nc.vector.bn_stats(out=st3[:, j, :], in_=G[:, 512 * j:512 * (j + 1)])
            mv3 = db.tile([32, 2], F32)
            nc.vector.bn_aggr(out=mv3, in_=st3)
            # SBUF-side partition-moving DMA views are unreliable; round-trip
            # through DRAM (DRAM-side views are fine).
            mvd = dr.tile([32, 2], F32, tag="mvd")
            w_mv = nc.sync.dma_start(out=mvd, in_=mv3)
            mvT = db.tile([2, 32], F32)
            r_mv = nc.sync.dma_start(out=mvT, in_=mvd.rearrange("p two -> two p"))
            tile.add_dep_helper(r_mv.ins, w_mv.ins, reason="mvd RAW")
            sq2 = db.tile([2, 32], F32)
            nc.vector.tensor_mul(sq2, mvT, mvT)
            r1 = db.tile([2, 2], F32)
            nc.vector.tensor_reduce(out=r1, in_=mvT.rearrange(
                "p (c g) -> p c g", c=2), axis=mybir.AxisListType.X, op=ALU.add)
            r2 = db.tile([2, 2], F32)
            nc.vector.tensor_reduce(out=r2, in_=sq2.rearrange(
                "p (c g) -> p c g", c=2), axis=mybir.AxisListType.X, op=ALU.add)
            rr = db.tile([1, 6], F32)
            nc.sync.dma_start(out=rr[:, 0:2], in_=r1[0:1, :])
            nc.sync.dma_start(out=rr[:, 2:4], in_=r1[1:2, :])
            nc.sync.dma_start(out=rr[:, 4:6], in_=r2[0:1, :])
            mn = db.tile([1, 2], F32)
            nc.vector.tensor_scalar_mul(mn, rr[:, 0:2], 1.0 / 16)
            q3 = db.tile([1, 2], F32)
            nc.vector.tensor_add(q3, rr[:, 2:4], rr[:, 4:6])
            q4 = db.tile([1, 2], F32)
            nc.vector.tensor_scalar_mul(q4, q3, 1.0 / 16)
            mn2 = db.tile([1, 2], F32)
            nc.vector.tensor_mul(mn2, mn, mn)
            vr = db.tile([1, 2], F32)
            nc.vector.tensor_sub(vr, q4, mn2)
            sd3 = db.tile([1, 2], F32)
            nc.scalar.activation(sd3, vr, AF.Sqrt, bias=epst)
            rs3 = db.tile([1, 2], F32)
            nc.vector.reciprocal(rs3, sd3)
            scl3 = db.tile([1, 2], F32)
            nc.vector.tensor_mul(scl3, gsb[:, 192:194], rs3)
            sh03 = db.tile([1, 2], F32)
            nc.vector.tensor_mul(sh03, mn, scl3)
            sh3 = db.tile([1, 2], F32)
            nc.vector.tensor_sub(sh3, besb[:, 192:194], sh03)
            ssT = db.tile([1, 4], F32)   # [scl0, scl1, sh0, sh1]
            nc.vector.tensor_copy(ssT[:, 0:2], scl3)
            nc.vector.tensor_copy(ssT[:, 2:4], sh3)
            ssd = dr.tile([1, 4], F32, tag="ssd")
            w_ss = nc.sync.dma_start(out=ssd, in_=ssT)
            SS = db.tile([2, 2], F32)
            r_ss = nc.sync.dma_start(out=SS, in_=ssd.rearrange(
                "o (j c) -> (o c) j", j=2))
            tile.add_dep_helper(r_ss.ins, w_ss.ins, reason="ssd RAW")
            B32 = db.tile([2, 32], F32)
            nc.sync.dma_start(out=B32, in_=b32f_in)
            B32h = db.tile([32, 2], F16)
            nc.sync.dma_start(out=B32h, in_=b32h_in)
            ps32 = sps.tile([32, 2], F32, tag="s")
            nc.tensor.matmul(ps32, B32, SS, start=True, stop=True)
            sb32 = db.tile([32, 2], F32)
            nc.scalar.copy(sb32, ps32)

            # normalized features, fp16 fabric
            F = db.tile([32, 4096], F32)
            nc.scalar.activation(F, G, AF.Relu,
                                 bias=sb32[:, 1:2], scale=sb32[:, 0:1])
            Fh = db.tile([32, 4096], F16)           # -2 * p~
            nc.vector.tensor_scalar_mul(Fh, F, -2.0)
            fhd = dr.tile([32, 4096], F16, tag="fhd")
            w_fh = nc.sync.dma_start(out=fhd, in_=Fh)
            # 4*|p~|^2 exactly: (2p~)^2 split hi+lo fp16, summed over the 16
            # components per channel by one accumulating PE matmul pair
            # (fp16 products are exact in the fp32 accumulator).
            Q32 = db.tile([32, 4096], F32)
            nc.vector.tensor_mul(Q32, Fh, Fh)
            Qhi = db.tile([32, 4096], F16)
            nc.vector.tensor_copy(Qhi, Q32)
            Qlo = db.tile([32, 4096], F16)   # f16 read upcasts exactly
            nc.vector.tensor_sub(Qlo, Q32, Qhi)
            sq2 = db.tile([2, 4096], F32)
            for j in range(8):
                pq = sps.tile([2, 512], F32, tag="s", name=f"sq{j}")
                nc.tensor.matmul(pq, B32h, Qhi[:, 512 * j:512 * (j + 1)],
                                 start=True, stop=False)
                nc.tensor.matmul(pq, B32h, Qlo[:, 512 * j:512 * (j + 1)],
                                 start=False, stop=True)
                if j % 2 == 0:
                    nc.scalar.copy(sq2[:, 512 * j:512 * (j + 1)], pq)
                else:
                    nc.vector.tensor_copy(sq2[:, 512 * j:512 * (j + 1)], pq)
            w_sq = nc.sync.dma_start(
                out=sqFd.rearrange("o (c n) -> (o c) n", c=2), in_=sq2)
            # partition-spread via DRAM (plain SBUF APs; grouped views only on
            # DRAM dims — the SBUF-side grouped view races with its writers)
            sqT4 = db.tile([128, 64], F32)
            r_spread = nc.sync.dma_start(
                out=sqT4, in_=sqFd.rearrange("o (p f) -> (o p) f", p=128))
            tile.add_dep_helper(r_spread.ins, w_sq.ins, reason="sqFd RAW")
            sqT = db.tile([128, 64], F32)           # |p~|^2 (x0.25 exact)
            nc.vector.tensor_scalar_mul(sqT, sqT4, 0.25)
            shi = db.tile([128, 64], F16)
            nc.vector.tensor_copy(shi, sqT)
            shi32 = db.tile([128, 64], F32)
            nc.vector.tensor_copy(shi32, shi)
            rlo = db.tile([128, 64], F32)
            nc.vector.tensor_sub(rlo, sqT, shi32)
            slo = db.tile([128, 64], F16)
            nc.vector.tensor_copy(slo, rlo)
            shid = dr.tile([128, 64], F16, tag="shid")
            slod = dr.tile([128, 64], F16, tag="slod")
            nc.sync.dma_start(out=shid, in_=shi)
            nc.sync.dma_start(out=slod, in_=slo)
            # assemble extended rhs rows in DRAM (p-rows straight from fhd:
            # global col order is [c0 block | c1 block])
            rd_wp = []
            rd_wp.append(nc.sync.dma_start(out=Rd[0:16, 0:4096],
                                           in_=fhd[0:16, :]))
            rd_wp.append(nc.sync.dma_start(out=Rd[0:16, 4096:8192],
                                           in_=fhd[16:32, :]))
            rd_wp.append(nc.sync.dma_start(out=Rd[0:16, 8192:8192 + 4096],
                                           in_=fhd[0:16, :]))
            rd_wp.append(nc.sync.dma_start(out=Rd[0:16, 8192 + 4096:REXT],
                                           in_=fhd[16:32, 0:512]))
            for w in rd_wp:
                tile.add_dep_helper(w.ins, w_fh.ins, reason="fhd RAW")
            rd_ws = []
            rd_ws.append(nc.gpsimd.dma_start(
                out=Rd[16:17, 0:8192],
                in_=shid.rearrange("p f -> (p f)").unsqueeze(0)))
            rd_ws.append(nc.gpsimd.dma_start(
                out=Rd[17:18, 0:8192],
                in_=slod.rearrange("p f -> (p f)").unsqueeze(0)))
            rd_ws.append(nc.gpsimd.dma_start(
                out=Rd[16:17, 8192:REXT],
                in_=shid[0:72, :].rearrange("p f -> (p f)").unsqueeze(0)))
            rd_ws.append(nc.gpsimd.dma_start(
                out=Rd[17:18, 8192:REXT],
                in_=slod[0:72, :].rearrange("p f -> (p f)").unsqueeze(0)))

            # per-core band windows (dynamic offsets keyed on device id).
            # Dynamic-offset reads are not seen by the dep tracker — add
            # explicit edges on every Rd/sqFd writer.
            pid = nc.sync.partition_id()
            off0 = pid * 512
            off1 = pid * 512 + 4096
            Rb = db.tile([128, 2 * BAND], F16)
            for b in (0, 32, 64, 96):
                # p-rows early (overlap the sq pipeline), sq rows later
                r0p = nc.sync.dma_start(out=Rb[b:b + 16, 0:BAND],
                                        in_=Rd[0:16, ds(off0, BAND)])
                r1p = nc.sync.dma_start(out=Rb[b:b + 16, BAND:2 * BAND],
                                        in_=Rd[0:16, ds(off1, BAND)])
                r0s = nc.sync.dma_start(out=Rb[b + 16:b + 18, 0:BAND],
                                        in_=Rd[16:18, ds(off0, BAND)])
                r1s = nc.sync.dma_start(out=Rb[b + 16:b + 18, BAND:2 * BAND],
                                        in_=Rd[16:18, ds(off1, BAND)])
                for w in rd_wp:
                    tile.add_dep_helper(r0p.ins, w.ins, reason="Rd p RAW")
                    tile.add_dep_helper(r1p.ins, w.ins, reason="Rd p RAW")
                for w in rd_ws:
                    tile.add_dep_helper(r0s.ins, w.ins, reason="Rd sq RAW")
                    tile.add_dep_helper(r1s.ins, w.ins, reason="Rd sq RAW")
            sqOwn = db.tile([1, 1024], F32)
            ro0 = nc.sync.dma_start(out=sqOwn[:, 0:512],
                                    in_=sqFd[:, ds(off0, 512)])
            ro1 = nc.sync.dma_start(out=sqOwn[:, 512:1024],
                                    in_=sqFd[:, ds(off1, 512)])
            tile.add_dep_helper(ro0.ins, w_sq.ins, reason="sqFd RAW")
            tile.add_dep_helper(ro1.ins, w_sq.ins, reason="sqFd RAW")

            # lhsT [18, 1024] x 4 row strips (p~ own, ones for sq rows)
            L = db.tile([128, 1024], F16)
            # memset wants f32: write two packed fp16(1.0) = bits 0x3C003C00
            one2 = float(np.frombuffer(np.uint32(0x3C003C00).tobytes(),
                                       np.float32)[0])
            nc.vector.memset(L.bitcast(F32), one2)  # sq rows stay ones
            for b in (0, 32, 64, 96):
                nc.vector.tensor_scalar_mul(L[b:b + 16, 0:512],
                                            Rb[b:b + 16, 0:512], -0.5)
                nc.vector.tensor_scalar_mul(L[b:b + 16, 512:1024],
                                            Rb[b:b + 16, BAND:BAND + 512], -0.5)

            # bias[:, t] = sq_i for m-tile t rows (+ eps)
            psb = sps.tile([128, 8], F32, tag="s", name="psb")
            for t in range(8):
                nc.tensor.matmul(psb[:, t:t + 1],
                                 sqOwn[:, 128 * t:128 * (t + 1)], ones1,
                                 start=True, stop=True)
            biasT = db.tile([128, 8], F32)   # sqFd holds 4*sq -> x0.25 + eps
            nc.vector.tensor_scalar(biasT, psb, 0.25, EPS2,
                                    op0=ALU.mult, op1=ALU.add)

          # distance loop: 8 m-tiles x 9 band tiles, 4-way PE row packing
          with tc.tile_pool(name="dps", bufs=2, space="PSUM") as dps:
            for t in range(8):
                cb0 = 0 if t < 4 else BAND
                stage = stg.tile([128, BAND], F16, tag="stage")
                for ch in range(3):          # 4+4+1 psum chunks
                    nts = range(4 * ch, min(4 * ch + 4, 9))
                    ps = dps.tile([128, 2048], F32, tag="dp",
                                  name=f"dp{t}_{ch}")
                    for i, u in enumerate(nts):
                        b = 32 * ((t * 9 + u) % 4)
                        nc.tensor.matmul(ps[:, 512 * i:512 * (i + 1)],
                                         L[b:b + 18, 128 * t:128 * (t + 1)],
                                         Rb[b:b + 18, cb0 + 512 * u:cb0 + 512 * (u + 1)],
                                         start=True, stop=True,
                                         tile_position=(b, 0))
                    w = 512 * len(nts)
                    nc.scalar.activation(
                        stage[:, 2048 * ch:2048 * ch + w], ps[:, 0:w],
                        AF.Sqrt, bias=biasT[:, t:t + 1])
                nc.sync.dma_start(out=out[128 * t:128 * (t + 1), :], in_=stage)
    nc.finalize()
    return nc


def _prep_inputs(x, ws_, gs, bes):
    """Per-core numpy input dicts."""
    xp = np.pad(x[0], ((0, 0), (5, 5), (2, 3))).astype(np.float32)
    w0 = ws_[0]
    w0T = np.ascontiguousarray(
        w0.transpose(2, 3, 1, 0).reshape(27, 64)).astype(np.float32)
    wp, wsg = {}, {}
    for l in (1, 2):
        w = ws_[l]
        wp[l] = np.ascontiguousarray(np.stack(
            [np.concatenate([w[:, :, 0, p].T, w[:, :, 1, p].T], 0)
             for p in range(3)])).astype(np.float32)
        wsg[l] = np.ascontiguousarray(np.stack(
            [w[:, :, 2, p].T for p in range(3)])).astype(np.float32)
    # conv3 patch-direct weights: [18 streams, 128=(half,ci), 32=(c,py,px)]
    w3 = ws_[3]
    w3c = np.zeros((18, 128, 32), np.float32)
    for iwyp in range(3):
        for wx in range(6):
            s = iwyp * 6 + wx
            for half in (0, 1):
                wy = 2 * iwyp + half
                for py in range(4):
                    ky = wy - py
                    if not 0 <= ky <= 2:
                        continue
                    for px in range(4):
                        kx = wx - px
                        if not 0 <= kx <= 2:
                            continue
                        for c in range(2):
                            w3c[s, half * 64:(half + 1) * 64,
                                c * 16 + py * 4 + px] = w3[c, :, ky, kx]
    b32f = np.zeros((2, 32), np.float32)
    b32f[0, 0:16] = 1.0
    b32f[1, 16:32] = 1.0
    b32h = np.zeros((32, 2), np.float16)
    b32h[0:16, 0] = 1.0
    b32h[16:32, 1] = 1.0
    g_all = np.concatenate([np.asarray(g, np.float32).ravel() for g in gs]
                           ).reshape(1, 194)
    be_all = np.concatenate([np.asarray(b, np.float32).ravel() for b in bes]
                            ).reshape(1, 194)
    in_maps = []
    for k in range(NCORES):
        col = np.empty((27, ROWS, WP), np.float32)
        for dy in range(3):
            for dx in range(3):
                for ci in range(3):
                    r0 = 32 * k + dy
                    col[(dy * 3 + dx) * 3 + ci] = xp[ci, r0:r0 + ROWS, dx:dx + WP]
        mask = np.zeros((8, WP), np.float32)
        for i, r in enumerate([0, 1, 2, 3, 36, 37, 38, 39]):
            ir = 32 * k - 4 + r
            if 0 <= ir < 256:
                mask[i, 1:257] = 1.0
        in_maps.append(dict(
            x0=np.ascontiguousarray(col.reshape(27, YFREE)),
            w0T=w0T, wp1=wp[1], ws1=wsg[1], wp2=wp[2], ws2=wsg[2],
            w3c=w3c, b32f=b32f, b32h=b32h, g_all=g_all, be_all=be_all,
            mask8=np.ascontiguousarray(mask.reshape(1, 8 * WP))))
    return in_maps


def kernel(x, w0, b0, g0, be0, w1, b1, g1, be1, w2, b2, g2, be2,
           w3, b3, g3, be3):
    # conv bias b_i cancels exactly inside BatchNorm (mean absorbs it); unused.
    if "nc" not in _CACHE:
        _CACHE["nc"] = build()
    nc = _CACHE["nc"]
    in_maps = _prep_inputs(
        np.asarray(x, np.float32),
        [np.asarray(w, np.float32) for w in (w0, w1, w2, w3)],
        (g0, g1, g2, g3), (be0, be1, be2, be3))
    res = run_bass_kernel_spmd(nc, in_maps, list(range(NCORES)))
    D = np.zeros((8192, 8192), np.float32)
    for k in range(NCORES):
        o = np.asarray(res.results[k]["out"], dtype=np.float32)
        for t in range(8):
            r0 = 512 * k + 128 * t if t < 4 else 4096 + 512 * k + 128 * (t - 4)
            base = (0 if t < 4 else 4096) + 512 * k
            for u in range(9):
                gc = (base + 512 * u) % 8192
                D[r0:r0 + 128, gc:gc + 512] = o[128 * t:128 * (t + 1),
                                                512 * u:512 * (u + 1)]
    Dt = np.ascontiguousarray(D.T)
    np.maximum(D, Dt, out=D)
    np.fill_diagonal(D, 0.0)
    return D


# revision 57
# speedup vs baseline: 1.2676x; 1.2676x over previous
"""TRN2 Bass kernel for nn_KNN_model (conv stack + pairwise patch distances).

Strategy (8 NeuronCores, SPMD):
  - Convs sharded over H: each core computes a 40-row slab (32 owned + 4 halo
    each side) through conv+BN+ReLU layers in float32r (TF32-like) on PE.
    3x3 conv = 6 matmul streams per tile: 3 K=128 pairs (top+mid tap rows via a
    partition-shifted slab copy) + 3 K=64 singles (bottom tap row).
  - BN stats (layers 0-2): per-core partial (mean, var) via bn_stats/bn_aggr,
    tiny AllGather + PE ones-matmul reduce, applied fused in one ACT pass.
  - conv3 emitted directly in patch layout: one K=2304 matmul chain (18 paired
    K=128 streams over the 6x6 stride-4 receptive field) -> [32=(c,py,px), 512]
    raw patch components per core.
  - Raw conv3 output AllGathered once ([8,32,512]); BN3 global stats computed
    locally post-gather (no separate stats collective).
  - Distance phase in fp16: p~=fp16(feat), sq=|p~|^2 split hi+lo fp16 so
    D2 = sq_i + sq_j - 2 p~.p~ stays PSD to ~1e-5.  lhsT rows [p~,1,1],
    rhs rows [-2p~, sq_hi, sq_lo]; per-partition ACT bias adds sq_i + eps and
    fuses sqrt + PSUM->SBUF + fp16 cast in ONE pass.  K=18 matmuls packed 4-way
    into PE row groups.
  - Symmetry: each row block computes only a 4608-wide wrapped column band
    (everything else is the transpose of some band entry). Host mirrors.
"""
import numpy as np
import ml_dtypes
import concourse.bacc as bacc
import concourse.bass as bass
import concourse.tile as tile
from concourse import mybir
from concourse.bass import ds
from concourse.bass_utils import run_bass_kernel_spmd

F32 = mybir.dt.float32
F32R = mybir.dt.float32r
F16 = mybir.dt.float16
AF = mybir.ActivationFunctionType
ALU = mybir.AluOpType

NCORES = 8
WP = 258            # padded row width (256 + 2 pad cols)
ROWS = 40           # ext slab rows per core (32 owned + 4 halo each side)
LEAD = 4            # lead margin so tap offsets never go negative
HROWS = 42          # slab rows + 1 pad row top/bottom
HFREE = LEAD + HROWS * WP + 4   # 10844
YFREE = ROWS * WP   # 10320
EPS = 1e-5
EPS2 = 2e-4         # d^2 floor bias before sqrt (abs err ~0.014 at d=0)
GOFF = [0, 64, 128, 192]        # g/be packing offsets per layer
COUT = [64, 64, 64, 2]
BAND = 4608         # symmetric band width (9 x 512)
REXT = 8192 + BAND  # extended rhs width for wraparound

_CACHE = {}


def _conv_tiles(s0=0, s1=YFREE):
    out, s = [], s0
    while s < s1:
        L = min(512, s1 - s)
        out.append((s, L))
        s += L
    return out


def build():
    nc = bacc.Bacc(trn_type="TRN2", num_devices=NCORES)
    x0 = nc.dram_tensor("x0", [27, YFREE], F32, kind="ExternalInput").ap()
    w0T = nc.dram_tensor("w0T", [27, 64], F32, kind="ExternalInput").ap()
    wp_in, ws_in = {}, {}
    for l in (1, 2):
        co = COUT[l]
        wp_in[l] = nc.dram_tensor(f"wp{l}", [3, 128, co], F32, kind="ExternalInput").ap()
        ws_in[l] = nc.dram_tensor(f"ws{l}", [3, 64, co], F32, kind="ExternalInput").ap()
    w3c_in = nc.dram_tensor("w3c", [18, 128, 32], F32, kind="ExternalInput").ap()
    b32f_in = nc.dram_tensor("b32f", [2, 32], F32, kind="ExternalInput").ap()
    b32h_in = nc.dram_tensor("b32h", [32, 2], F16, kind="ExternalInput").ap()
    b32v_in = nc.dram_tensor("b32v", [32, 2], F32, kind="ExternalInput").ap()
    g3c_in = nc.dram_tensor("g3c", [2, 1], F32, kind="ExternalInput").ap()
    be3c_in = nc.dram_tensor("be3c", [2, 1], F32, kind="ExternalInput").ap()
    g_all = nc.dram_tensor("g_all", [1, 194], F32, kind="ExternalInput").ap()
    be_all = nc.dram_tensor("be_all", [1, 194], F32, kind="ExternalInput").ap()
    mask8 = nc.dram_tensor("mask8", [1, 8 * WP], F32, kind="ExternalInput").ap()
    out = nc.dram_tensor("out", [1024, BAND], F16, kind="ExternalOutput").ap()

    TILES = {0: _conv_tiles(WP, 39 * WP),
             1: _conv_tiles(2 * WP, 38 * WP),
             2: _conv_tiles(3 * WP, 37 * WP)}

    with tile.TileContext(nc) as tc:
      with tc.tile_pool(name="pers", bufs=1) as pers, \
           tc.tile_pool(name="dr", bufs=1, space="DRAM") as dr:
        gsb = pers.tile([1, 194], F32)
        nc.sync.dma_start(out=gsb, in_=g_all)
        besb = pers.tile([1, 194], F32)
        nc.sync.dma_start(out=besb, in_=be_all)
        ones1 = pers.tile([1, 1], F32)
        nc.vector.memset(ones1, 1.0)
        ones8 = pers.tile([8, 1], F32)
        nc.vector.memset(ones8, 0.125)   # 1/8 for mean-of-cores matmul
        epst = pers.tile([1, 1], F32)
        nc.vector.memset(epst, EPS)

        def bn_finish(l, C, regions, bnps, sbp):
            """Cross-core BN: partial stats -> AllGather -> scale/shift [C,1]."""
            n = sum(r.shape[1] if r.ndim == 3 else 1 for r in regions)
            st = sbp.tile([C, n, 6], F32, tag=f"st{l}")
            i = 0
            for ap in regions:
                k = ap.shape[1] if ap.ndim == 3 else 1
                o = st[:, i:i + k, :] if ap.ndim == 3 else st[:, i, :]
                nc.vector.bn_stats(out=o, in_=ap)
                i += k
            mvt = sbp.tile([C, 2], F32, tag=f"mv{l}")
            nc.vector.bn_aggr(out=mvt, in_=st)
            sti = dr.tile([C, 2], F32, tag=f"sti{l}")
            sto = dr.tile([NCORES, C, 2], F32, tag=f"sto{l}")
            nc.gpsimd.dma_start(out=sti, in_=mvt)
            nc.gpsimd.collective_compute(
                "AllGather", ALU.bypass,
                replica_groups=[list(range(NCORES))],
                ins=[sti.opt()], outs=[sto.opt()])
            G = sbp.tile([8, 2 * C], F32, tag=f"G{l}")
            nc.sync.dma_start(out=G, in_=sto.rearrange("k c two -> k (c two)"))
            Gv = G.rearrange("p (c two) -> p c two", two=2)
            m2 = sbp.tile([8, C], F32, tag=f"m2{l}")
            nc.vector.tensor_mul(m2, Gv[:, :, 0], Gv[:, :, 0])
            pavg = bnps.tile([1, 2 * C], F32, tag="bn")
            nc.tensor.matmul(pavg, ones8, G, start=True, stop=True)
            pavg2 = bnps.tile([1, C], F32, tag="bn")
            nc.tensor.matmul(pavg2, ones8, m2, start=True, stop=True)
            A1 = sbp.tile([1, 2 * C], F32, tag=f"A1{l}")
            nc.scalar.copy(A1, pavg)
            A2 = sbp.tile([1, C], F32, tag=f"A2{l}")
            nc.scalar.copy(A2, pavg2)
            A1v = A1.rearrange("p (c two) -> p c two", two=2)
            am, av = A1v[:, :, 0], A1v[:, :, 1]
            t1 = sbp.tile([1, C], F32, tag=f"t1{l}")
            nc.vector.tensor_mul(t1, am, am)       # E[m]^2
            t2 = sbp.tile([1, C], F32, tag=f"t2{l}")
            nc.vector.tensor_sub(t2, A2, t1)       # Var(means)
            t3 = sbp.tile([1, C], F32, tag=f"t3{l}")
            nc.vector.tensor_add(t3, t2, av)       # + E[var] = total var
            sd = sbp.tile([1, C], F32, tag=f"sd{l}")
            nc.scalar.activation(sd, t3, AF.Sqrt, bias=epst)
            rs = sbp.tile([1, C], F32, tag=f"rs{l}")
            nc.vector.reciprocal(rs, sd)
            off = GOFF[l]
            scl = sbp.tile([1, C], F32, tag=f"scl{l}")
            nc.vector.tensor_mul(scl, gsb[:, off:off + C], rs)
            sh0 = sbp.tile([1, C], F32, tag=f"sh0{l}")
            nc.vector.tensor_mul(sh0, am, scl)
            sh = sbp.tile([1, C], F32, tag=f"sh{l}")
            nc.vector.tensor_sub(sh, besb[:, off:off + C], sh0)
            psc = bnps.tile([C, 1], F32, tag="bn")
            nc.tensor.matmul(psc, scl, ones1, start=True, stop=True)
            psh = bnps.tile([C, 1], F32, tag="bn")
            nc.tensor.matmul(psh, sh, ones1, start=True, stop=True)
            sbs = sbp.tile([C, 1], F32, tag=f"sbs{l}")
            nc.scalar.copy(sbs, psc)
            sbh = sbp.tile([C, 1], F32, tag=f"sbh{l}")
            nc.scalar.copy(sbh, psh)
            return sbs, sbh

        # ---------------- conv phase ----------------
        y3d = dr.tile([32, 512], F32, tag="y3d")
        with tc.tile_pool(name="cb", bufs=1) as cb, \
             tc.tile_pool(name="hp", bufs=2) as hp, \
             tc.tile_pool(name="cps", bufs=6, space="PSUM") as cps, \
             tc.tile_pool(name="bnps", bufs=2, space="PSUM") as bnps:
            x0t = cb.tile([27, YFREE], F32R)
            nc.gpsimd.dma_start(out=x0t, in_=x0)
            mskf = cb.tile([64, 8 * WP], F32)
            nc.gpsimd.dma_start(out=mskf, in_=mask8.partition_broadcast(64))
            mv_ = mskf.rearrange("p (r c) -> p r c", c=WP)
            w0 = cb.tile([27, 64], F32R)
            nc.gpsimd.dma_start(out=w0, in_=w0T)
            wpair, wsing = {}, {}
            for l in (1, 2):
                co = COUT[l]
                for p in range(3):
                    t = cb.tile([128, co], F32R, tag=f"twp{l}{p}")
                    nc.gpsimd.dma_start(out=t, in_=wp_in[l][p])
                    wpair[(l, p)] = t
                    t2 = cb.tile([64, co], F32R, tag=f"tws{l}{p}")
                    nc.gpsimd.dma_start(out=t2, in_=ws_in[l][p])
                    wsing[(l, p)] = t2
            w3sb = []
            for s in range(18):
                t = cb.tile([128, 32], F32R, tag=f"w3c{s}")
                nc.gpsimd.dma_start(out=t, in_=w3c_in[s])
                w3sb.append(t)

            def finish_layer(l, y):
                """BN + ReLU + mask + padded f32r slab, chunked so the next
                conv's matmuls can start before the whole pass finishes."""
                yv = y.rearrange("p (r c) -> p r c", c=WP)
                regs = [yv[:, r, 1:257] for r in range(4, 36)]
                sbs, sbh = bn_finish(l, 64, regs, bnps, cb)
                h = hp.tile([128, HFREE], F32R, tag="h")
                T0 = LEAD + WP
                nc.vector.memset(h[0:64, 0:T0].bitcast(F32), 0.0)
                nc.vector.memset(h[0:64, T0 + YFREE:HFREE].bitcast(F32), 0.0)
                CH = 10   # slab rows per chunk
                for c0 in range(0, ROWS, CH):
                    a = T0 + c0 * WP
                    nc.scalar.activation(h[0:64, a:a + CH * WP],
                                         y[:, c0 * WP:(c0 + CH) * WP],
                                         AF.Relu, bias=sbh, scale=sbs)
                    hvv = h[0:64, a:a + CH * WP].rearrange(
                        "p (r c) -> p r c", c=WP)
                    if c0 == 0:
                        nc.vector.tensor_mul(hvv[:, 0:4, :], hvv[:, 0:4, :],
                                             mv_[:, 0:4, :])
                    if c0 == 30:
                        nc.vector.tensor_mul(hvv[:, 6:10, :], hvv[:, 6:10, :],
                                             mv_[:, 4:8, :])
                    hcv = h[0:64, a:a + CH * WP].rearrange(
                        "p (r c) -> p c r", c=WP)
                    nc.vector.memset(hcv[:, 0, :].bitcast(F32), 0.0)
                    nc.vector.memset(hcv[:, 257, :].bitcast(F32), 0.0)
                # bottom half = top shifted one row, chunked to follow ACT
                nc.vector.memset(h[64:128, 0:LEAD].bitcast(F32), 0.0)
                nc.vector.memset(h[64:128, LEAD + YFREE:HFREE].bitcast(F32), 0.0)
                for c0 in range(0, ROWS, CH):
                    d0 = LEAD + c0 * WP
                    nc.vector.tensor_copy(h[64:128, d0:d0 + CH * WP],
                                          h[0:64, d0 + WP:d0 + WP + CH * WP])
                return h

            # conv0 (im2col input, K=27, one stream)
            if True:
                y = cb.tile([64, YFREE], F32, tag="y")
                for (s, L) in TILES[0]:
                    ps = cps.tile([64, 512], F32, tag="cps")
                    nc.tensor.matmul(ps[:, 0:L], w0, x0t[:, s:s + L],
                                     start=True, stop=True)
                    nc.scalar.copy(y[:, s:s + L], ps[:, 0:L])
                h = finish_layer(0, y)

            # conv1, conv2 (6 streams: 3 pairs K=128 + 3 singles K=64)
            GROUP = 6
            for l in (1, 2):
                y = cb.tile([64, YFREE], F32, tag="y")
                for g0 in range(0, len(TILES[l]), GROUP):
                    grp = TILES[l][g0:g0 + GROUP]
                    pss = [cps.tile([64, 512], F32, tag="cps", name=f"cps{g0}_{i}")
                           for i in range(len(grp))]
                    for p in range(3):
                        for ps, (s, L) in zip(pss, grp):
                            o = LEAD + 516 + s + p - 1
                            nc.tensor.matmul(ps[:, 0:L], wsing[(l, p)],
                                             h[0:64, o:o + L],
                                             start=(p == 0), stop=False)
                    for p in range(3):
                        for ps, (s, L) in zip(pss, grp):
                            o = LEAD + s + p - 1
                            nc.tensor.matmul(ps[:, 0:L], wpair[(l, p)],
                                             h[0:128, o:o + L],
                                             start=False, stop=(p == 2))
                    for ps, (s, L) in zip(pss, grp):
                        nc.scalar.copy(y[:, s:s + L], ps[:, 0:L])
                h = finish_layer(l, y)

            # conv3 direct to patch layout: out[(c,py,px),(gy,gx)] via K=2304
            # (18 paired K=128 streams over the 6x6 stride-4 window)
            c3ps = cps.tile([64, 512], F32, tag="cps", name="c3ps")[0:32, :]
            si = 0
            for iwyp in range(3):
                wyp = 2 * iwyp
                for wx in range(6):
                    base = LEAD + (wyp + 4) * WP + wx
                    win = h[0:128, base:base + 8 * 4 * WP]
                    w1 = win.rearrange("p (gy r) -> p gy r", gy=8)
                    w2 = w1[:, :, 0:256]
                    rhs = w2.rearrange("p gy (gx s) -> p gy gx s", s=4)[:, :, :, 0:1]
                    nc.tensor.matmul(c3ps, w3sb[si], rhs,
                                     start=(si == 0), stop=(si == 17))
                    si += 1
            y3l = cb.tile([32, 512], F32, tag="y3l")
            nc.scalar.copy(y3l, c3ps)
            nc.gpsimd.dma_start(out=y3d, in_=y3l)

        # ---------------- gather raw conv3 output ----------------
        gath3 = dr.tile([NCORES, 32, 512], F32, tag="gath3")
        nc.gpsimd.collective_compute(
            "AllGather", ALU.bypass,
            replica_groups=[list(range(NCORES))],
            ins=[y3d.opt()], outs=[gath3.opt()])

        # ---------------- BN3 + feature build + distance ----------------
        Rd = dr.tile([18, REXT], F16, tag="Rd")     # extended rhs rows in DRAM
        sqFd = dr.tile([1, 8192], F32, tag="sqFd")  # per-patch |p|^2 (fp32)
        with tc.tile_pool(name="db", bufs=1) as db, \
             tc.tile_pool(name="stg", bufs=2) as stg:
          with tc.tile_pool(name="sps", bufs=2, space="PSUM") as sps:
            G = db.tile([32, 4096], F32)
            nc.sync.dma_start(out=G.rearrange("p (k n) -> p k n", k=8),
                              in_=gath3.rearrange("k p n -> p k n"))

            # global BN3 stats from gathered raw data
            st3 = db.tile([32, 8, 6], F32)
            for j in range(8):
                nc.vector.bn_stats(out=st3[:, j, :], in_=G[:, 512 * j:512 * (j + 1)])
            # per-partition stats [32, (mean, var, mean^2)]; one PE matmul
            # against a block-ones lhsT sums the 16-partition channel groups
            # (replaces the DRAM-transpose round trips).
            mvm = db.tile([32, 3], F32)
            nc.vector.bn_aggr(out=mvm[:, 0:2], in_=st3)
            nc.vector.tensor_mul(mvm[:, 2:3], mvm[:, 0:1], mvm[:, 0:1])
            B32v = db.tile([32, 2], F32)
            nc.sync.dma_start(out=B32v, in_=b32v_in)
            g3c = db.tile([2, 1], F32)
            nc.sync.dma_start(out=g3c, in_=g3c_in)
            be3c = db.tile([2, 1], F32)
            nc.sync.dma_start(out=be3c, in_=be3c_in)
            eps2t = db.tile([2, 1], F32)
            nc.vector.memset(eps2t, EPS)
            ps3 = sps.tile([2, 3], F32, tag="s", name="ps3")
            nc.tensor.matmul(ps3, B32v, mvm, start=True, stop=True)
            s3 = db.tile([2, 3], F32)
            nc.scalar.copy(s3, ps3)
            mn = db.tile([2, 1], F32)
            nc.vector.tensor_scalar_mul(mn, s3[:, 0:1], 1.0 / 16)
            q3 = db.tile([2, 1], F32)
            nc.vector.tensor_add(q3, s3[:, 1:2], s3[:, 2:3])
            q4 = db.tile([2, 1], F32)
            nc.vector.tensor_scalar_mul(q4, q3, 1.0 / 16)
            mn2 = db.tile([2, 1], F32)
            nc.vector.tensor_mul(mn2, mn, mn)
            vr = db.tile([2, 1], F32)
            nc.vector.tensor_sub(vr, q4, mn2)
            sd3 = db.tile([2, 1], F32)
            nc.scalar.activation(sd3, vr, AF.Sqrt, bias=eps2t)
            rs3 = db.tile([2, 1], F32)
            nc.vector.reciprocal(rs3, sd3)
            scl3 = db.tile([2, 1], F32)
            nc.vector.tensor_mul(scl3, g3c, rs3)
            sh03 = db.tile([2, 1], F32)
            nc.vector.tensor_mul(sh03, mn, scl3)
            sh3 = db.tile([2, 1], F32)
            nc.vector.tensor_sub(sh3, be3c, sh03)
            SS = db.tile([2, 2], F32)
            nc.vector.tensor_copy(SS[:, 0:1], scl3)
            nc.vector.tensor_copy(SS[:, 1:2], sh3)
            B32 = db.tile([2, 32], F32)
            nc.sync.dma_start(out=B32, in_=b32f_in)
            B32h = db.tile([32, 2], F16)
            nc.sync.dma_start(out=B32h, in_=b32h_in)
            ps32 = sps.tile([32, 2], F32, tag="s")
            nc.tensor.matmul(ps32, B32, SS, start=True, stop=True)
            sb32 = db.tile([32, 2], F32)
            nc.scalar.copy(sb32, ps32)

            # normalized features, fp16 fabric
            F = db.tile([32, 4096], F32)
            nc.scalar.activation(F, G, AF.Relu,
                                 bias=sb32[:, 1:2], scale=sb32[:, 0:1])
            Fh = db.tile([32, 4096], F16)           # -2 * p~
            nc.vector.tensor_scalar_mul(Fh, F, -2.0)
            # 4*|p~|^2 exactly: (2p~)^2 split hi+lo fp16, summed over the 16
            # components per channel by one accumulating PE matmul pair
            # (fp16 products are exact in the fp32 accumulator).
            Q32 = db.tile([32, 4096], F32)
            nc.vector.tensor_mul(Q32, Fh, Fh)
            Qhi = db.tile([32, 4096], F16)
            nc.vector.tensor_copy(Qhi, Q32)
            Qlo = db.tile([32, 4096], F16)   # f16 read upcasts exactly
            nc.vector.tensor_sub(Qlo, Q32, Qhi)
            sq2 = db.tile([2, 4096], F32)
            for j in range(8):
                pq = sps.tile([2, 512], F32, tag="s", name=f"sq{j}")
                nc.tensor.matmul(pq, B32h, Qhi[:, 512 * j:512 * (j + 1)],
                                 start=True, stop=False)
                nc.tensor.matmul(pq, B32h, Qlo[:, 512 * j:512 * (j + 1)],
                                 start=False, stop=True)
                if j % 2 == 0:
                    nc.scalar.copy(sq2[:, 512 * j:512 * (j + 1)], pq)
                else:
                    nc.vector.tensor_copy(sq2[:, 512 * j:512 * (j + 1)], pq)
            w_sq = nc.sync.dma_start(
                out=sqFd.rearrange("o (c n) -> (o c) n", c=2), in_=sq2)
            # partition-spread via DRAM (plain SBUF APs; grouped views only on
            # DRAM dims — the SBUF-side grouped view races with its writers)
            sqT4 = db.tile([128, 64], F32)
            r_spread = nc.sync.dma_start(
                out=sqT4, in_=sqFd.rearrange("o (p f) -> (o p) f", p=128))
            tile.add_dep_helper(r_spread.ins, w_sq.ins, reason="sqFd RAW")
            sqT = db.tile([128, 64], F32)           # |p~|^2 (x0.25 exact)
            nc.vector.tensor_scalar_mul(sqT, sqT4, 0.25)
            shi = db.tile([128, 64], F16)
            nc.vector.tensor_copy(shi, sqT)
            shi32 = db.tile([128, 64], F32)
            nc.vector.tensor_copy(shi32, shi)
            rlo = db.tile([128, 64], F32)
            nc.vector.tensor_sub(rlo, sqT, shi32)
            slo = db.tile([128, 64], F16)
            nc.vector.tensor_copy(slo, rlo)
            shid = dr.tile([128, 64], F16, tag="shid")
            slod = dr.tile([128, 64], F16, tag="slod")
            nc.sync.dma_start(out=shid, in_=shi)
            nc.sync.dma_start(out=slod, in_=slo)
            # assemble extended rhs rows in DRAM (p-rows straight from fhd:
            # global col order is [c0 block | c1 block])
            rd_wp = []
            rd_wp.append(nc.sync.dma_start(out=Rd[0:16, 0:4096],
                                           in_=Fh[0:16, :]))
            rd_wp.append(nc.sync.dma_start(out=Rd[0:16, 4096:8192],
                                           in_=Fh[16:32, :]))
            rd_wp.append(nc.sync.dma_start(out=Rd[0:16, 8192:8192 + 4096],
                                           in_=Fh[0:16, :]))
            rd_wp.append(nc.sync.dma_start(out=Rd[0:16, 8192 + 4096:REXT],
                                           in_=Fh[16:32, 0:512]))
            rd_ws = []
            rd_ws.append(nc.gpsimd.dma_start(
                out=Rd[16:17, 0:8192],
                in_=shid.rearrange("p f -> (p f)").unsqueeze(0)))
            rd_ws.append(nc.gpsimd.dma_start(
                out=Rd[17:18, 0:8192],
                in_=slod.rearrange("p f -> (p f)").unsqueeze(0)))
            rd_ws.append(nc.gpsimd.dma_start(
                out=Rd[16:17, 8192:REXT],
                in_=shid[0:72, :].rearrange("p f -> (p f)").unsqueeze(0)))
            rd_ws.append(nc.gpsimd.dma_start(
                out=Rd[17:18, 8192:REXT],
                in_=slod[0:72, :].rearrange("p f -> (p f)").unsqueeze(0)))

            # per-core band windows (dynamic offsets keyed on device id).
            # Dynamic-offset reads are not seen by the dep tracker — add
            # explicit edges on every Rd/sqFd writer.
            pid = nc.sync.partition_id()
            off0 = pid * 512
            off1 = pid * 512 + 4096
            Rb = db.tile([128, 2 * BAND], F16)
            for b in (0, 32, 64, 96):
                # p-rows early (overlap the sq pipeline), sq rows later
                r0p = nc.sync.dma_start(out=Rb[b:b + 16, 0:BAND],
                                        in_=Rd[0:16, ds(off0, BAND)])
                r1p = nc.sync.dma_start(out=Rb[b:b + 16, BAND:2 * BAND],
                                        in_=Rd[0:16, ds(off1, BAND)])
                r0s = nc.sync.dma_start(out=Rb[b + 16:b + 18, 0:BAND],
                                        in_=Rd[16:18, ds(off0, BAND)])
                r1s = nc.sync.dma_start(out=Rb[b + 16:b + 18, BAND:2 * BAND],
                                        in_=Rd[16:18, ds(off1, BAND)])
                for w in rd_wp:
                    tile.add_dep_helper(r0p.ins, w.ins, reason="Rd p RAW")
                    tile.add_dep_helper(r1p.ins, w.ins, reason="Rd p RAW")
                for w in rd_ws:
                    tile.add_dep_helper(r0s.ins, w.ins, reason="Rd sq RAW")
                    tile.add_dep_helper(r1s.ins, w.ins, reason="Rd sq RAW")
            sqOwn = db.tile([1, 1024], F32)
            ro0 = nc.sync.dma_start(out=sqOwn[:, 0:512],
                                    in_=sqFd[:, ds(off0, 512)])
            ro1 = nc.sync.dma_start(out=sqOwn[:, 512:1024],
                                    in_=sqFd[:, ds(off1, 512)])
            tile.add_dep_helper(ro0.ins, w_sq.ins, reason="sqFd RAW")
            tile.add_dep_helper(ro1.ins, w_sq.ins, reason="sqFd RAW")

            # lhsT [18, 1024] x 4 row strips (p~ own, ones for sq rows)
            L = db.tile([128, 1024], F16)
            # memset wants f32: write two packed fp16(1.0) = bits 0x3C003C00
            one2 = float(np.frombuffer(np.uint32(0x3C003C00).tobytes(),
                                       np.float32)[0])
            nc.vector.memset(L.bitcast(F32), one2)  # sq rows stay ones
            for b in (0, 32, 64, 96):
                nc.vector.tensor_scalar_mul(L[b:b + 16, 0:512],
                                            Rb[b:b + 16, 0:512], -0.5)
                nc.vector.tensor_scalar_mul(L[b:b + 16, 512:1024],
                                            Rb[b:b + 16, BAND:BAND + 512], -0.5)

            # bias[:, t] = sq_i for m-tile t rows (+ eps)
            psb = sps.tile([128, 8], F32, tag="s", name="psb")
            for t in range(8):
                nc.tensor.matmul(psb[:, t:t + 1],
                                 sqOwn[:, 128 * t:128 * (t + 1)], ones1,
                                 start=True, stop=True)
            biasT = db.tile([128, 8], F32)   # sqFd holds 4*sq -> x0.25 + eps
            nc.vector.tensor_scalar(biasT, psb, 0.25, EPS2,
                                    op0=ALU.mult, op1=ALU.add)

          # distance loop: 8 m-tiles x 9 band tiles, 4-way PE row packing
          with tc.tile_pool(name="dps", bufs=2, space="PSUM") as dps:
            for t in range(8):
                cb0 = 0 if t < 4 else BAND
                stage = stg.tile([128, BAND], F16, tag="stage")
                for ch in range(3):          # 4+4+1 psum chunks
                    nts = range(4 * ch, min(4 * ch + 4, 9))
                    ps = dps.tile([128, 2048], F32, tag="dp",
                                  name=f"dp{t}_{ch}")
                    for i, u in enumerate(nts):
                        b = 32 * ((t * 9 + u) % 4)
                        nc.tensor.matmul(ps[:, 512 * i:512 * (i + 1)],
                                         L[b:b + 18, 128 * t:128 * (t + 1)],
                                         Rb[b:b + 18, cb0 + 512 * u:cb0 + 512 * (u + 1)],
                                         start=True, stop=True,
                                         tile_position=(b, 0))
                    w = 512 * len(nts)
                    nc.scalar.activation(
                        stage[:, 2048 * ch:2048 * ch + w], ps[:, 0:w],
                        AF.Sqrt, bias=biasT[:, t:t + 1])
                nc.sync.dma_start(out=out[128 * t:128 * (t + 1), :], in_=stage)
    nc.finalize()
    return nc


def _prep_inputs(x, ws_, gs, bes):
    """Per-core numpy input dicts."""
    xp = np.pad(x[0], ((0, 0), (5, 5), (2, 3))).astype(np.float32)
    w0 = ws_[0]
    w0T = np.ascontiguousarray(
        w0.transpose(2, 3, 1, 0).reshape(27, 64)).astype(np.float32)
    wp, wsg = {}, {}
    for l in (1, 2):
        w = ws_[l]
        wp[l] = np.ascontiguousarray(np.stack(
            [np.concatenate([w[:, :, 0, p].T, w[:, :, 1, p].T], 0)
             for p in range(3)])).astype(np.float32)
        wsg[l] = np.ascontiguousarray(np.stack(
            [w[:, :, 2, p].T for p in range(3)])).astype(np.float32)
    # conv3 patch-direct weights: [18 streams, 128=(half,ci), 32=(c,py,px)]
    w3 = ws_[3]
    w3c = np.zeros((18, 128, 32), np.float32)
    for iwyp in range(3):
        for wx in range(6):
            s = iwyp * 6 + wx
            for half in (0, 1):
                wy = 2 * iwyp + half
                for py in range(4):
                    ky = wy - py
                    if not 0 <= ky <= 2:
                        continue
                    for px in range(4):
                        kx = wx - px
                        if not 0 <= kx <= 2:
                            continue
                        for c in range(2):
                            w3c[s, half * 64:(half + 1) * 64,
                                c * 16 + py * 4 + px] = w3[c, :, ky, kx]
    b32f = np.zeros((2, 32), np.float32)
    b32f[0, 0:16] = 1.0
    b32f[1, 16:32] = 1.0
    b32h = np.zeros((32, 2), np.float16)
    b32h[0:16, 0] = 1.0
    b32h[16:32, 1] = 1.0
    b32v = b32h.astype(np.float32)
    g3c = np.asarray(gs[3], np.float32).reshape(2, 1)
    be3c = np.asarray(bes[3], np.float32).reshape(2, 1)
    g_all = np.concatenate([np.asarray(g, np.float32).ravel() for g in gs]
                           ).reshape(1, 194)
    be_all = np.concatenate([np.asarray(b, np.float32).ravel() for b in bes]
                            ).reshape(1, 194)
    in_maps = []
    for k in range(NCORES):
        col = np.empty((27, ROWS, WP), np.float32)
        for dy in range(3):
            for dx in range(3):
                for ci in range(3):
                    r0 = 32 * k + dy
                    col[(dy * 3 + dx) * 3 + ci] = xp[ci, r0:r0 + ROWS, dx:dx + WP]
        mask = np.zeros((8, WP), np.float32)
        for i, r in enumerate([0, 1, 2, 3, 36, 37, 38, 39]):
            ir = 32 * k - 4 + r
            if 0 <= ir < 256:
                mask[i, 1:257] = 1.0
        in_maps.append(dict(
            x0=np.ascontiguousarray(col.reshape(27, YFREE)),
            w0T=w0T, wp1=wp[1], ws1=wsg[1], wp2=wp[2], ws2=wsg[2],
            w3c=w3c, b32f=b32f, b32h=b32h, b32v=b32v, g3c=g3c, be3c=be3c,
            g_all=g_all, be_all=be_all,
            mask8=np.ascontiguousarray(mask.reshape(1, 8 * WP))))
    return in_maps


def kernel(x, w0, b0, g0, be0, w1, b1, g1, be1, w2, b2, g2, be2,
           w3, b3, g3, be3):
    # conv bias b_i cancels exactly inside BatchNorm (mean absorbs it); unused.
    if "nc" not in _CACHE:
        _CACHE["nc"] = build()
    nc = _CACHE["nc"]
    in_maps = _prep_inputs(
        np.asarray(x, np.float32),
        [np.asarray(w, np.float32) for w in (w0, w1, w2, w3)],
        (g0, g1, g2, g3), (be0, be1, be2, be3))
    res = run_bass_kernel_spmd(nc, in_maps, list(range(NCORES)))
    D = np.zeros((8192, 8192), np.float32)
    for k in range(NCORES):
        o = np.asarray(res.results[k]["out"], dtype=np.float32)
        for t in range(8):
            r0 = 512 * k + 128 * t if t < 4 else 4096 + 512 * k + 128 * (t - 4)
            base = (0 if t < 4 else 4096) + 512 * k
            for u in range(9):
                gc = (base + 512 * u) % 8192
                D[r0:r0 + 128, gc:gc + 512] = o[128 * t:128 * (t + 1),
                                                512 * u:512 * (u + 1)]
    Dt = np.ascontiguousarray(D.T)
    np.maximum(D, Dt, out=D)
    np.fill_diagonal(D, 0.0)
    return D
